# revision 47
# baseline (speedup 1.0000x reference)
"""TRN2 Bass kernel for nn_CharModel (segment-mean over char ranges + pos embedding).

Strategy (pure data-parallel over batch, 8 cores x 4 batches):
linear fp16 streaming + PE assignment-matmul segment reduce.

  - Valid words tile [0, seq_len) contiguously, so the segment-mean is a
    banded sparse matmul: out[w, :] = sum_{chars p of w} recip_w * feats[p, :].
    Instead of per-word indirect gathers (small descriptors, Pool-engine
    descriptor generation, DVE fold trees), feats is streamed LINEARLY into
    SBUF as [128, 64, 768] fp16 (char-in-tile on partitions) with ~9 KB
    contiguous descriptors, and the reduce runs on the otherwise-idle PE:
      psum[128 words, 768] += A_tile^T @ feats_tile
    where A_tile[p, m] = (wordid[char p] == block_word m) * (q*recip).
  - Word blocks are tile-ALIGNED: greedy runs of 3-4 char tiles whose word
    span stays <= 128 on every core (unified SPMD geometry; batches are
    clustered to core-slots by boundary similarity first). Every char tile
    is matmul'd exactly once. A word straddling a block boundary yields two
    partial sums; the host adds them after dequantization.
  - A_tile matrices are built on-device, one DVE tensor_scalar each:
    (iota == wid_rel[p]) * rq[p], per-partition scalars from tiny host
    tables. rq[p] = q * recip(word(p)) applies each word's 1/len and the
    int8 output scale inside the matmul.
  - The pos embedding is a one-hot matmul accumulated into the same PSUM
    group: psum += OH^T @ pos_table, OH[v, m] = q * (pos[w0+m] == v), set
    only in the first block containing the word. Invalid words keep pos id
    0 and pos_table row 0 is zeros, matching the reference exactly.
  - ACT drains each [128, 768] PSUM block to an int8 SBUF staging tile
    (values bounded by q*(|mean|+|pos|) <= 118 < 127 by construction of q);
    grouped SWDGE writes (4 blocks each) move it to a partition-major DRAM
    tensor. The host transposes, divides by q, and accumulates block rows.
  - Scheduling, learned from traces: blocks interleave round-robin across
    the 4 batches so PE idle stays sub-microsecond (no HAM re-throttle);
    ~28 dummy iota matmuls warm the PE HAM ramp (half->full rate) before
    the real stream; pos matmuls are emitted with a 2-block lag so PSUM
    recycling hides the drain round-trip; const tables load before feats on
    the same queue so their descriptors aren't stuck behind 12.6 MB; output
    writes ride the independent SWDGE queue.
  - HBM traffic per core: 12.6 MB fp16 feats in + 2.3 MB int8 out + ~0.6 MB
    tables, all full-rate descriptors, vs 17.4 MB with gather descriptors
    and pad slots in the v1 gather/fold design (80.9 us -> 59.3 us).
"""

import numpy as np

B, S, W, D, PV = 32, 2048, 512, 768, 64
N_CORES = 8
BPC = B // N_CORES          # batches per core
P = 128
NT = S // P                 # char tiles per batch (16)
WB = W // P                 # word blocks per batch (4)
NBLK = BPC * WB             # psum blocks per core (16)

LAST_RESULTS = None         # BassKernelResults of the most recent run (for test.py)


def _run_spmd(nc, in_maps, core_ids):
    """Indirection point so tests can swap in a simulator."""
    from concourse.bass_utils import run_bass_kernel_spmd
    return run_bass_kernel_spmd(nc, in_maps, core_ids)


def _word_ranges(word_lens, pos, seq_len):
    """Replicate the reference's starts/ends/valid computation in numpy."""
    wl = np.asarray(word_lens, np.int64)
    po = np.asarray(pos, np.int64)
    sl = np.asarray(seq_len, np.int64)
    b, w = wl.shape
    j = np.arange(w)
    next_start = np.concatenate([wl[:, 1:], np.zeros((b, 1), np.int64)], axis=1)
    is_last = (j[None, :] == w - 1) | (next_start == 0)
    starts = wl
    ends = np.where(is_last, sl[:, None], next_start)
    valid = (wl != 0) | (j[None, :] == 0)
    lens = np.where(valid, np.maximum(ends - starts, 0), 0)
    denom = np.maximum(ends - starts, 1).astype(np.float64)
    recip = np.where(valid & (lens > 0), 1.0 / denom, 0.0).astype(np.float32)
    return starts, lens, recip, po


def _numpy_fallback(feats, pos_table, word_lens, pos, seq_len):
    feats = np.asarray(feats, np.float32)
    pos_table = np.asarray(pos_table, np.float32)
    starts, lens, recip, po = _word_ranges(word_lens, pos, seq_len)
    out = np.zeros((feats.shape[0], po.shape[1], feats.shape[2]), np.float32)
    for b in range(out.shape[0]):
        for w in range(out.shape[1]):
            L = int(lens[b, w])
            if L > 0:
                s = int(starts[b, w])
                out[b, w] = feats[b, s:s + L].sum(axis=0) * recip[b, w]
        out[b] += pos_table[po[b]]
    return out


def _concourse_importable():
    try:
        import concourse.bass  # noqa: F401
        return True
    except ImportError:
        import sys
        for p in ("/opt/trn_rl_repo", "/root/.axon_site/_ro/trn_rl_repo"):
            if p not in sys.path:
                sys.path.append(p)
        try:
            import concourse.bass  # noqa: F401
            return True
        except ImportError:
            return False


def _prepare(feats, pos_table_np, starts, lens, recip, po):
    """Host-side layout: wordid map, unified tile windows, per-core tables."""
    amax_f = float(np.abs(feats).max())
    amax_p = float(np.abs(pos_table_np).max()) if pos_table_np.size else 0.0
    q = float(np.float16(127.0 / (amax_f + amax_p + 1e-6)))

    # wordid[b, c] = word owning char c (valid words tile [0, seq_len))
    wordid = np.full((B, S), -10000, np.int32)
    for b in range(B):
        for w in range(W):
            L = int(lens[b, w])
            if L > 0:
                s0 = int(starts[b, w])
                wordid[b, s0:min(s0 + L, S)] = w

    # Assign batches to (core, slot) so batches sharing a slot have similar
    # word-block boundaries: block geometry is unified over cores (SPMD), so
    # clustering by the middle boundary shrinks the unions. Sort by start of
    # word 256 and deal into the 4 slots.
    border = np.array([int(starts[b, W // 2]) for b in range(B)])
    order = np.argsort(border, kind="stable")
    slot_of = np.zeros((N_CORES, BPC), np.int64)
    for g in range(BPC):
        grp = order[g * N_CORES:(g + 1) * N_CORES]
        for c in range(N_CORES):
            slot_of[c, g] = grp[c]

    # number of valid chars per batch (valid words tile [0, nval))
    nval = np.asarray([(wordid[b] >= 0).sum() for b in range(B)])

    # per-slot tile count: max over cores of tiles holding any valid char
    his = np.zeros(BPC, np.int64)
    for bl in range(BPC):
        for c in range(N_CORES):
            bg = int(slot_of[c, bl])
            his[bl] = max(his[bl], -(-int(nval[bg]) // P))

    def span(bg, a, b):
        """word span of chars [a*P, b*P) in batch bg (valid chars only)."""
        nv = int(nval[bg])
        if a * P >= nv:
            return 0
        last = min(b * P, nv) - 1
        return int(wordid[bg, last]) - int(wordid[bg, a * P]) + 1

    # greedy unified blocks per bl: consecutive tiles while every core's
    # word span stays <= 128 (a single tile always fits: span <= 128)
    per_bl = {bl: [] for bl in range(BPC)}
    for bl in range(BPC):
        a = 0
        while a < int(his[bl]):
            b = a + 1
            while b < int(his[bl]) and all(
                    span(int(slot_of[c, bl]), a, b + 1) <= P
                    for c in range(N_CORES)):
                b += 1
            per_bl[bl].append((a, b))
            a = b
    # Interleave blocks round-robin over bl in PAIRS (one ~6-tile DMA chunk
    # feeds two blocks): the PE consumes each batch's work slightly faster
    # than DMA delivers it, and strict bl-major order accumulates that drift
    # into >3us PE idle cliffs at batch boundaries, HAM-throttling the PE to
    # half rate. Round-robin keeps the gaps sub-microsecond.
    blocks_flat = []            # (bl, a, b)
    nrounds = max((len(v) + 1) // 2 for v in per_bl.values())
    for r in range(nrounds):
        for bl in range(BPC):
            for blk in per_bl[bl][2 * r:2 * r + 2]:
                blocks_flat.append((bl,) + blk)
    nrows = [max(span(int(slot_of[c, bl]), a, b) for c in range(N_CORES))
             for (bl, a, b) in blocks_flat]
    nblk = len(blocks_flat)

    iota = np.tile(np.arange(P, dtype=np.float16), (P, 1))
    in_maps = []
    w0s_all = []
    for core in range(N_CORES):
        bgs = [int(slot_of[core, bl]) for bl in range(BPC)]
        fh = feats[bgs].reshape(-1, D).astype(np.float16)         # [8192, 768]
        feats2 = np.ascontiguousarray(
            fh.reshape(BPC * NT, P, D).transpose(1, 0, 2))        # [128, 64, 768]

        # block start word per (core, block); W if the block has no valid
        # chars for this core
        w0s = np.full(nblk, W, np.int64)
        for blk, (bl, a, b) in enumerate(blocks_flat):
            bg = bgs[bl]
            if a * P < int(nval[bg]):
                w0s[blk] = int(wordid[bg, a * P])
        w0s_all.append(w0s)

        # wid_rel per global tile slot, relative to the OWNING block's w0
        widrel = np.full((P, BPC * NT), -20000.0, np.float32)
        for blk, (bl, a, b) in enumerate(blocks_flat):
            bg = bgs[bl]
            for t in range(a, b):
                widrel[:, bl * NT + t] = (
                    wordid[bg, t * P:(t + 1) * P] - w0s[blk]).astype(np.float32)

        rq = np.zeros((P, BPC * NT), np.float32)
        for bl in range(BPC):
            bg = bgs[bl]
            wi = wordid[bg]
            r = np.where(wi >= 0, recip[bg][np.clip(wi, 0, W - 1)] * q, 0.0)
            rq[:, bl * NT:(bl + 1) * NT] = r.reshape(NT, P).T

        # pos one-hot: q at (pos[w], w - w0) in the FIRST block containing w
        oh = np.zeros((PV, nblk * P), np.float16)
        for blk, (bl, a, b) in enumerate(blocks_flat):
            bg = bgs[bl]
            nv = int(nval[bg])
            if a * P >= nv:
                continue
            w0 = int(w0s[blk])
            whigh = int(wordid[bg, min(b * P, nv) - 1])
            wlo = w0
            if a > 0 and a * P - 1 < nv and int(wordid[bg, a * P - 1]) == w0:
                wlo = w0 + 1          # split word: pos set in previous block
            for w in range(wlo, whigh + 1):
                oh[po[bg, w], blk * P + (w - w0)] = np.float16(q)

        in_maps.append({
            "feats2": feats2,
            "pos_tab": pos_table_np.astype(np.float16),
            "iota": iota,
            "widrel": widrel,
            "rq": rq,
            "oh": oh,
        })
    return blocks_flat, nrows, his, nblk, in_maps, q, slot_of, w0s_all


def _build_nc(blocks_flat, nrows, his, nblk):
    from concourse import bacc, mybir
    import concourse.tile as tile

    nc = bacc.Bacc("TRN2", target_bir_lowering=False, debug=False)
    t_feats = nc.dram_tensor("feats2", [P, BPC * NT, D], mybir.dt.float16,
                             kind="ExternalInput")
    t_pos = nc.dram_tensor("pos_tab", [PV, D], mybir.dt.float16,
                           kind="ExternalInput")
    t_iota = nc.dram_tensor("iota", [P, P], mybir.dt.float16,
                            kind="ExternalInput")
    t_widrel = nc.dram_tensor("widrel", [P, BPC * NT], mybir.dt.float32,
                              kind="ExternalInput")
    t_rq = nc.dram_tensor("rq", [P, BPC * NT], mybir.dt.float32,
                          kind="ExternalInput")
    t_oh = nc.dram_tensor("oh", [PV, nblk * P], mybir.dt.float16,
                          kind="ExternalInput")
    # partition-major output: row (p, blk) holds word w0[blk] + p of its
    # block; the host transposes for free. Grouped writes (4 blocks each)
    # amortize the ~1us SWDGE descriptor-generation per DMA.
    t_out = nc.dram_tensor("out", [P, nblk, D], mybir.dt.int8,
                           kind="ExternalOutput")

    with tile.TileContext(nc) as tc:
        PSB = 4     # psum pipeline depth (each buf is bank-aligned: 2 banks)
        with (
            tc.tile_pool(name="const", bufs=1) as cpool,
            tc.tile_pool(name="psum", bufs=PSB, space="PSUM") as ppool,
        ):
            iota_sb = cpool.tile([P, P], mybir.dt.float16)
            widrel_sb = cpool.tile([P, BPC * NT], mybir.dt.float32)
            rq_sb = cpool.tile([P, BPC * NT], mybir.dt.float32)
            oh_sb = cpool.tile([PV, nblk * P], mybir.dt.float16)
            pos_sb = cpool.tile([PV, D], mybir.dt.float16)
            feats_sb = cpool.tile([P, BPC * NT, D], mybir.dt.float16)
            a_sb = cpool.tile([P, BPC * NT * P], mybir.dt.float16)
            osb_all = cpool.tile([P, nblk, D], mybir.dt.int8)

            # consts FIRST on the same queue as feats: their descriptors must
            # not queue behind 12.6 MB of feats in the DMA engines. oh/pos
            # come first so the prefetched pos matmuls can warm the PE early.
            nc.sync.dma_start(out=iota_sb[:], in_=t_iota[:])
            nc.sync.dma_start(out=oh_sb[:], in_=t_oh[:])
            nc.sync.dma_start(out=pos_sb[:], in_=t_pos[:])
            nc.sync.dma_start(out=widrel_sb[:], in_=t_widrel[:])
            nc.sync.dma_start(out=rq_sb[:], in_=t_rq[:])

            # feats chunks aligned to block PAIRS, same round-robin order as
            # the PE consumes blocks (~6-tile chunks amortize DGE overhead)
            seen = {bl: 0 for bl in range(BPC)}
            chunks = []
            for bl, a, b in blocks_flat:
                if b > seen[bl]:
                    chunks.append((bl, seen[bl], b))
                    seen[bl] = b
            merged = []
            for bl, a, b in chunks:
                if (len(merged) > 1 and merged[-1][0] == bl
                        and merged[-1][2] == a):
                    merged[-1] = (bl, merged[-1][1], b)
                else:
                    merged.append((bl, a, b))
            for bl, a, b in merged:
                nc.sync.dma_start(
                    out=feats_sb[:, bl * NT + a:bl * NT + b, :],
                    in_=t_feats[:, bl * NT + a:bl * NT + b, :])

            # assignment matrices: A[p, m] = (iota[m] == wid_rel[p]) * rq[p]
            for bl in range(BPC):
                for t in range(int(his[bl])):
                    s = bl * NT + t
                    nc.vector.tensor_scalar(
                        out=a_sb[:, s * P:(s + 1) * P], in0=iota_sb[:, :],
                        scalar1=widrel_sb[:, s:s + 1],
                        scalar2=rq_sb[:, s:s + 1],
                        op0=mybir.AluOpType.is_equal, op1=mybir.AluOpType.mult)

            # HAM warm-up: the PE runs at K=4/8 (half rate) until it has been
            # continuously busy for several microseconds, and any idle gap
            # resets the ramp. Dummy back-to-back matmuls (iota x iota into a
            # scratch PSUM bank) start as soon as the first const lands, so
            # the real stream begins at full rate.
            warm = ppool.tile([P, D], mybir.dt.float32, space="PSUM",
                              tag="ps")
            for _ in range(28):
                nc.tensor.matmul(out=warm[:, 0:128], lhsT=iota_sb[:, :],
                                 rhs=iota_sb[:, :], start=True, stop=True)

            # pos matmuls are prefetched ahead so the PE has work while a
            # block waits for its feats chunk
            psums = {}

            def start_block(blk):
                if blk >= nblk:
                    return
                psum = ppool.tile([P, D], mybir.dt.float32, space="PSUM",
                                  tag="ps")
                lhs = oh_sb[:, blk * P:(blk + 1) * P]
                nc.tensor.matmul(out=psum[:, 0:512], lhsT=lhs,
                                 rhs=pos_sb[:, 0:512], start=True, stop=False)
                nc.tensor.matmul(out=psum[:, 512:D], lhsT=lhs,
                                 rhs=pos_sb[:, 512:D], start=True, stop=False)
                psums[blk] = psum

            # emission lag 2: pos(k+2) is EMITTED after pairs(k), so by the
            # time the PE reaches it, drain(k-2)'s full round-trip (sem +
            # ACT copy + sem, ~2.7us) has completed under two blocks of real
            # work. Emitting earlier makes the PE sit out part of that
            # round-trip between blocks.
            wstart = [0]
            for blk in range(PSB - 2):
                start_block(blk)
            for blk, (bl, a, b) in enumerate(blocks_flat):
                psum = psums.pop(blk)
                for t in range(a, b):
                    s = bl * NT + t
                    asl = a_sb[:, s * P:(s + 1) * P]
                    last = t == b - 1
                    nc.tensor.matmul(out=psum[:, 0:512], lhsT=asl,
                                     rhs=feats_sb[:, s, 0:512],
                                     start=False, stop=last)
                    nc.tensor.matmul(out=psum[:, 512:D], lhsT=asl,
                                     rhs=feats_sb[:, s, 512:D],
                                     start=False, stop=last)
                nc.scalar.activation(out=osb_all[:, blk, :], in_=psum[:, :],
                                     func=mybir.ActivationFunctionType.Copy)
                if (blk % 4 == 3 and blk <= nblk - 5) or blk >= nblk - 2:
                    g = wstart[0]
                    eng = nc.scalar if blk == nblk - 1 else nc.gpsimd
                    eng.dma_start(out=t_out[:, g:blk + 1, :],
                                  in_=osb_all[:, g:blk + 1, :])
                    wstart[0] = blk + 1
                start_block(blk + PSB - 2)
    nc.finalize()
    return nc


def kernel(feats, pos_table, word_lens, pos, seq_len):
    global LAST_RESULTS
    feats = np.ascontiguousarray(np.asarray(feats, np.float32))
    pos_table_np = np.ascontiguousarray(np.asarray(pos_table, np.float32))
    starts, lens, recip, po = _word_ranges(word_lens, pos, seq_len)

    shapes_ok = (
        feats.shape == (B, S, D)
        and pos_table_np.shape == (PV, D)
        and po.shape == (B, W)
        and starts.shape == (B, W)
        and np.asarray(seq_len).shape == (B,)
        and int(po.max()) < PV and int(po.min()) >= 0
    )
    if not shapes_ok or not _concourse_importable():
        return _numpy_fallback(feats, pos_table, word_lens, pos, seq_len)

    blocks_flat, nrows, his, nblk, in_maps, q, slot_of, w0s_all = _prepare(
        feats, pos_table_np, starts, lens, recip, po)
    nc = _build_nc(blocks_flat, nrows, his, nblk)

    res = _run_spmd(nc, in_maps, list(range(N_CORES)))
    LAST_RESULTS = res

    # Accumulate block rows: words split across a block boundary contribute
    # partial sums from two blocks; unused rows are exact zeros.
    out = np.zeros((B, W, D), np.float32)
    for core in range(N_CORES):
        arr = np.asarray(res.results[core]["out"])     # [128, nblk, 768] int8
        dq = (arr.astype(np.float32) / q).transpose(1, 0, 2).reshape(-1, D)
        w0s = w0s_all[core]
        for blk, (bl, a, b) in enumerate(blocks_flat):
            bg = int(slot_of[core, bl])
            w0 = int(w0s[blk])
            if w0 >= W:
                continue
            n = min(int(nrows[blk]), W - w0)
            out[bg, w0:w0 + n] += dq[blk * P:blk * P + n]
    return out


# revision 48
# speedup vs baseline: 1.0413x; 1.0413x over previous
"""TRN2 Bass kernel for nn_CharModel (segment-mean over char ranges + pos embedding).

Strategy (pure data-parallel over batch, 8 cores x 4 batches):
linear fp16 streaming + PE assignment-matmul segment reduce.

  - Valid words tile [0, seq_len) contiguously, so the segment-mean is a
    banded sparse matmul: out[w, :] = sum_{chars p of w} recip_w * feats[p, :].
    Instead of per-word indirect gathers (small descriptors, Pool-engine
    descriptor generation, DVE fold trees), feats is streamed LINEARLY into
    SBUF as [128, 64, 768] fp16 (char-in-tile on partitions) with ~9 KB
    contiguous descriptors, and the reduce runs on the otherwise-idle PE:
      psum[128 words, 768] += A_tile^T @ feats_tile
    where A_tile[p, m] = (wordid[char p] == block_word m) * (q*recip).
  - Word blocks are tile-ALIGNED: greedy runs of 3-4 char tiles whose word
    span stays <= 128 on every core (unified SPMD geometry; batches are
    clustered to core-slots by boundary similarity first). Every char tile
    is matmul'd exactly once. A word straddling a block boundary yields two
    partial sums; the host adds them after dequantization.
  - A_tile matrices are built on-device, one DVE tensor_scalar each:
    (iota == wid_rel[p]) * rq[p], per-partition scalars from tiny host
    tables. rq[p] = q * recip(word(p)) applies each word's 1/len and the
    int8 output scale inside the matmul.
  - The pos embedding is a one-hot matmul accumulated into the same PSUM
    group: psum += OH^T @ pos_table, OH[v, m] = q * (pos[w0+m] == v), set
    only in the first block containing the word. Invalid words keep pos id
    0 and pos_table row 0 is zeros, matching the reference exactly.
  - ACT drains each [128, 768] PSUM block to an int8 SBUF staging tile
    (values bounded by q*(|mean|+|pos|) <= 118 < 127 by construction of q);
    grouped SWDGE writes (4 blocks each) move it to a partition-major DRAM
    tensor. The host transposes, divides by q, and accumulates block rows.
  - Scheduling, learned from traces: blocks interleave round-robin across
    the 4 batches so PE idle stays sub-microsecond (no HAM re-throttle);
    ~28 dummy iota matmuls warm the PE HAM ramp (half->full rate) before
    the real stream; pos matmuls are emitted with a 2-block lag so PSUM
    recycling hides the drain round-trip; const tables load before feats on
    the same queue so their descriptors aren't stuck behind 12.6 MB; output
    writes ride the independent SWDGE queue.
  - HBM traffic per core: 12.6 MB fp16 feats in + 2.3 MB int8 out + ~0.6 MB
    tables, all full-rate descriptors, vs 17.4 MB with gather descriptors
    and pad slots in the v1 gather/fold design (80.9 us -> 59.3 us).
"""

import numpy as np

B, S, W, D, PV = 32, 2048, 512, 768, 64
N_CORES = 8
BPC = B // N_CORES          # batches per core
P = 128
NT = S // P                 # char tiles per batch (16)
WB = W // P                 # word blocks per batch (4)
NBLK = BPC * WB             # psum blocks per core (16)

LAST_RESULTS = None         # BassKernelResults of the most recent run (for test.py)


def _run_spmd(nc, in_maps, core_ids):
    """Indirection point so tests can swap in a simulator."""
    from concourse.bass_utils import run_bass_kernel_spmd
    return run_bass_kernel_spmd(nc, in_maps, core_ids)


def _word_ranges(word_lens, pos, seq_len):
    """Replicate the reference's starts/ends/valid computation in numpy."""
    wl = np.asarray(word_lens, np.int64)
    po = np.asarray(pos, np.int64)
    sl = np.asarray(seq_len, np.int64)
    b, w = wl.shape
    j = np.arange(w)
    next_start = np.concatenate([wl[:, 1:], np.zeros((b, 1), np.int64)], axis=1)
    is_last = (j[None, :] == w - 1) | (next_start == 0)
    starts = wl
    ends = np.where(is_last, sl[:, None], next_start)
    valid = (wl != 0) | (j[None, :] == 0)
    lens = np.where(valid, np.maximum(ends - starts, 0), 0)
    denom = np.maximum(ends - starts, 1).astype(np.float64)
    recip = np.where(valid & (lens > 0), 1.0 / denom, 0.0).astype(np.float32)
    return starts, lens, recip, po


def _numpy_fallback(feats, pos_table, word_lens, pos, seq_len):
    feats = np.asarray(feats, np.float32)
    pos_table = np.asarray(pos_table, np.float32)
    starts, lens, recip, po = _word_ranges(word_lens, pos, seq_len)
    out = np.zeros((feats.shape[0], po.shape[1], feats.shape[2]), np.float32)
    for b in range(out.shape[0]):
        for w in range(out.shape[1]):
            L = int(lens[b, w])
            if L > 0:
                s = int(starts[b, w])
                out[b, w] = feats[b, s:s + L].sum(axis=0) * recip[b, w]
        out[b] += pos_table[po[b]]
    return out


def _concourse_importable():
    try:
        import concourse.bass  # noqa: F401
        return True
    except ImportError:
        import sys
        for p in ("/opt/trn_rl_repo", "/root/.axon_site/_ro/trn_rl_repo"):
            if p not in sys.path:
                sys.path.append(p)
        try:
            import concourse.bass  # noqa: F401
            return True
        except ImportError:
            return False


def _prepare(feats, pos_table_np, starts, lens, recip, po):
    """Host-side layout: wordid map, unified tile windows, per-core tables."""
    amax_f = float(np.abs(feats).max())
    amax_p = float(np.abs(pos_table_np).max()) if pos_table_np.size else 0.0
    q = float(np.float16(127.0 / (amax_f + amax_p + 1e-6)))

    # wordid[b, c] = word owning char c (valid words tile [0, seq_len))
    wordid = np.full((B, S), -10000, np.int32)
    for b in range(B):
        for w in range(W):
            L = int(lens[b, w])
            if L > 0:
                s0 = int(starts[b, w])
                wordid[b, s0:min(s0 + L, S)] = w

    # Assign batches to (core, slot) so batches sharing a slot have similar
    # word-block boundaries: block geometry is unified over cores (SPMD), so
    # clustering by the middle boundary shrinks the unions. Sort by start of
    # word 256 and deal into the 4 slots.
    border = np.array([int(starts[b, W // 2]) for b in range(B)])
    order = np.argsort(border, kind="stable")
    slot_of = np.zeros((N_CORES, BPC), np.int64)
    for g in range(BPC):
        grp = order[g * N_CORES:(g + 1) * N_CORES]
        for c in range(N_CORES):
            slot_of[c, g] = grp[c]

    # number of valid chars per batch (valid words tile [0, nval))
    nval = np.asarray([(wordid[b] >= 0).sum() for b in range(B)])

    # per-slot tile count: max over cores of tiles holding any valid char
    his = np.zeros(BPC, np.int64)
    for bl in range(BPC):
        for c in range(N_CORES):
            bg = int(slot_of[c, bl])
            his[bl] = max(his[bl], -(-int(nval[bg]) // P))

    def span(bg, a, b):
        """word span of chars [a*P, b*P) in batch bg (valid chars only)."""
        nv = int(nval[bg])
        if a * P >= nv:
            return 0
        last = min(b * P, nv) - 1
        return int(wordid[bg, last]) - int(wordid[bg, a * P]) + 1

    # greedy unified blocks per bl: consecutive tiles while every core's
    # word span stays <= 128 (a single tile always fits: span <= 128)
    per_bl = {bl: [] for bl in range(BPC)}
    for bl in range(BPC):
        a = 0
        while a < int(his[bl]):
            b = a + 1
            while b < int(his[bl]) and all(
                    span(int(slot_of[c, bl]), a, b + 1) <= P
                    for c in range(N_CORES)):
                b += 1
            per_bl[bl].append((a, b))
            a = b
    # Interleave blocks round-robin over bl in PAIRS (one ~6-tile DMA chunk
    # feeds two blocks): the PE consumes each batch's work slightly faster
    # than DMA delivers it, and strict bl-major order accumulates that drift
    # into >3us PE idle cliffs at batch boundaries, HAM-throttling the PE to
    # half rate. Round-robin keeps the gaps sub-microsecond.
    blocks_flat = []            # (bl, a, b)
    nrounds = max((len(v) + 1) // 2 for v in per_bl.values())
    for r in range(nrounds):
        for bl in range(BPC):
            for blk in per_bl[bl][2 * r:2 * r + 2]:
                blocks_flat.append((bl,) + blk)
    nrows = [max(span(int(slot_of[c, bl]), a, b) for c in range(N_CORES))
             for (bl, a, b) in blocks_flat]
    nblk = len(blocks_flat)

    iota = np.tile(np.arange(P, dtype=np.float16), (P, 1))
    in_maps = []
    w0s_all = []
    for core in range(N_CORES):
        bgs = [int(slot_of[core, bl]) for bl in range(BPC)]
        fh = feats[bgs].reshape(-1, D).astype(np.float16)         # [8192, 768]
        feats2 = np.ascontiguousarray(
            fh.reshape(BPC * NT, P, D).transpose(1, 0, 2))        # [128, 64, 768]

        # block start word per (core, block); W if the block has no valid
        # chars for this core
        w0s = np.full(nblk, W, np.int64)
        for blk, (bl, a, b) in enumerate(blocks_flat):
            bg = bgs[bl]
            if a * P < int(nval[bg]):
                w0s[blk] = int(wordid[bg, a * P])
        w0s_all.append(w0s)

        # wid_rel per global tile slot, relative to the OWNING block's w0
        widrel = np.full((P, BPC * NT), -20000.0, np.float32)
        for blk, (bl, a, b) in enumerate(blocks_flat):
            bg = bgs[bl]
            for t in range(a, b):
                widrel[:, bl * NT + t] = (
                    wordid[bg, t * P:(t + 1) * P] - w0s[blk]).astype(np.float32)

        rq = np.zeros((P, BPC * NT), np.float32)
        for bl in range(BPC):
            bg = bgs[bl]
            wi = wordid[bg]
            r = np.where(wi >= 0, recip[bg][np.clip(wi, 0, W - 1)] * q, 0.0)
            rq[:, bl * NT:(bl + 1) * NT] = r.reshape(NT, P).T

        # pos one-hot: q at (pos[w], w - w0) in the FIRST block containing w
        oh = np.zeros((PV, nblk * P), np.float16)
        for blk, (bl, a, b) in enumerate(blocks_flat):
            bg = bgs[bl]
            nv = int(nval[bg])
            if a * P >= nv:
                continue
            w0 = int(w0s[blk])
            whigh = int(wordid[bg, min(b * P, nv) - 1])
            wlo = w0
            if a > 0 and a * P - 1 < nv and int(wordid[bg, a * P - 1]) == w0:
                wlo = w0 + 1          # split word: pos set in previous block
            for w in range(wlo, whigh + 1):
                oh[po[bg, w], blk * P + (w - w0)] = np.float16(q)

        in_maps.append({
            "feats2": feats2,
            "pos_tab": pos_table_np.astype(np.float16),
            "iota": iota,
            "widrel": widrel,
            "rq": rq,
            "oh": oh,
        })
    return blocks_flat, nrows, his, nblk, in_maps, q, slot_of, w0s_all


def _build_nc(blocks_flat, nrows, his, nblk):
    from concourse import bacc, mybir
    import concourse.tile as tile

    nc = bacc.Bacc("TRN2", target_bir_lowering=False, debug=False)
    t_feats = nc.dram_tensor("feats2", [P, BPC * NT, D], mybir.dt.float16,
                             kind="ExternalInput")
    t_pos = nc.dram_tensor("pos_tab", [PV, D], mybir.dt.float16,
                           kind="ExternalInput")
    t_iota = nc.dram_tensor("iota", [P, P], mybir.dt.float16,
                            kind="ExternalInput")
    t_widrel = nc.dram_tensor("widrel", [P, BPC * NT], mybir.dt.float32,
                              kind="ExternalInput")
    t_rq = nc.dram_tensor("rq", [P, BPC * NT], mybir.dt.float32,
                          kind="ExternalInput")
    t_oh = nc.dram_tensor("oh", [PV, nblk * P], mybir.dt.float16,
                          kind="ExternalInput")
    # partition-major output: row (p, blk) holds word w0[blk] + p of its
    # block; the host transposes for free. Grouped writes (4 blocks each)
    # amortize the ~1us SWDGE descriptor-generation per DMA.
    t_out = nc.dram_tensor("out", [P, nblk, D], mybir.dt.int8,
                           kind="ExternalOutput")

    with tile.TileContext(nc) as tc:
        PSB = 4     # psum pipeline depth (each buf is bank-aligned: 2 banks)
        with (
            tc.tile_pool(name="const", bufs=1) as cpool,
            tc.tile_pool(name="psum", bufs=PSB, space="PSUM") as ppool,
        ):
            iota_sb = cpool.tile([P, P], mybir.dt.float16)
            widrel_sb = cpool.tile([P, BPC * NT], mybir.dt.float32)
            rq_sb = cpool.tile([P, BPC * NT], mybir.dt.float32)
            oh_sb = cpool.tile([PV, nblk * P], mybir.dt.float16)
            pos_sb = cpool.tile([PV, D], mybir.dt.float16)
            feats_sb = cpool.tile([P, BPC * NT, D], mybir.dt.float16)
            a_sb = cpool.tile([P, BPC * NT * P], mybir.dt.float16)
            osb_all = cpool.tile([P, nblk, D], mybir.dt.int8)

            # consts FIRST on the same queue as feats: their descriptors must
            # not queue behind 12.6 MB of feats in the DMA engines. oh/pos
            # come first so the prefetched pos matmuls can warm the PE early.
            nc.sync.dma_start(out=iota_sb[:], in_=t_iota[:])
            nc.sync.dma_start(out=oh_sb[:], in_=t_oh[:])
            nc.sync.dma_start(out=pos_sb[:], in_=t_pos[:])
            nc.sync.dma_start(out=widrel_sb[:], in_=t_widrel[:])
            nc.sync.dma_start(out=rq_sb[:], in_=t_rq[:])

            # feats chunks aligned to block PAIRS, same round-robin order as
            # the PE consumes blocks (~6-tile chunks amortize DGE overhead)
            seen = {bl: 0 for bl in range(BPC)}
            chunks = []
            for bl, a, b in blocks_flat:
                if b > seen[bl]:
                    chunks.append((bl, seen[bl], b))
                    seen[bl] = b
            merged = []
            for bl, a, b in chunks:
                if (len(merged) > 1 and merged[-1][0] == bl
                        and merged[-1][2] == a):
                    merged[-1] = (bl, merged[-1][1], b)
                else:
                    merged.append((bl, a, b))
            for bl, a, b in merged:
                nc.sync.dma_start(
                    out=feats_sb[:, bl * NT + a:bl * NT + b, :],
                    in_=t_feats[:, bl * NT + a:bl * NT + b, :])

            # assignment matrices: A[p, m] = (iota[m] == wid_rel[p]) * rq[p]
            for bl in range(BPC):
                for t in range(int(his[bl])):
                    s = bl * NT + t
                    nc.vector.tensor_scalar(
                        out=a_sb[:, s * P:(s + 1) * P], in0=iota_sb[:, :],
                        scalar1=widrel_sb[:, s:s + 1],
                        scalar2=rq_sb[:, s:s + 1],
                        op0=mybir.AluOpType.is_equal, op1=mybir.AluOpType.mult)

            # HAM warm-up: the PE runs at K=4/8 (half rate) until it has been
            # continuously busy for several microseconds, and any idle gap
            # resets the ramp. Dummy back-to-back matmuls (iota x iota into a
            # scratch PSUM bank) start as soon as the first const lands, so
            # the real stream begins at full rate.
            warm = ppool.tile([P, D], mybir.dt.float32, space="PSUM",
                              tag="ps")
            for _ in range(32):
                nc.tensor.matmul(out=warm[:, 0:128], lhsT=iota_sb[:, :],
                                 rhs=iota_sb[:, :], start=True, stop=True)

            # pos matmuls are prefetched ahead so the PE has work while a
            # block waits for its feats chunk
            psums = {}

            def start_block(blk):
                if blk >= nblk:
                    return
                psum = ppool.tile([P, D], mybir.dt.float32, space="PSUM",
                                  tag="ps")
                lhs = oh_sb[:, blk * P:(blk + 1) * P]
                nc.tensor.matmul(out=psum[:, 0:512], lhsT=lhs,
                                 rhs=pos_sb[:, 0:512], start=True, stop=False)
                nc.tensor.matmul(out=psum[:, 512:D], lhsT=lhs,
                                 rhs=pos_sb[:, 512:D], start=True, stop=False)
                psums[blk] = psum

            # emission lag 2: pos(k+2) is EMITTED after pairs(k), so by the
            # time the PE reaches it, drain(k-2)'s full round-trip (sem +
            # ACT copy + sem, ~2.7us) has completed under two blocks of real
            # work. Emitting earlier makes the PE sit out part of that
            # round-trip between blocks.
            wstart = [0]
            for blk in range(PSB - 2):
                start_block(blk)
            for blk, (bl, a, b) in enumerate(blocks_flat):
                psum = psums.pop(blk)
                for t in range(a, b):
                    s = bl * NT + t
                    asl = a_sb[:, s * P:(s + 1) * P]
                    last = t == b - 1
                    nc.tensor.matmul(out=psum[:, 0:512], lhsT=asl,
                                     rhs=feats_sb[:, s, 0:512],
                                     start=False, stop=last)
                    nc.tensor.matmul(out=psum[:, 512:D], lhsT=asl,
                                     rhs=feats_sb[:, s, 512:D],
                                     start=False, stop=last)
                nc.scalar.activation(out=osb_all[:, blk, :], in_=psum[:, :],
                                     func=mybir.ActivationFunctionType.Copy)
                if (blk % 4 == 3 and blk <= nblk - 5) or blk >= nblk - 2:
                    g = wstart[0]
                    eng = nc.scalar if blk == nblk - 1 else nc.gpsimd
                    eng.dma_start(out=t_out[:, g:blk + 1, :],
                                  in_=osb_all[:, g:blk + 1, :])
                    wstart[0] = blk + 1
                start_block(blk + PSB - 2)
    nc.finalize()
    return nc


def kernel(feats, pos_table, word_lens, pos, seq_len):
    global LAST_RESULTS
    feats = np.ascontiguousarray(np.asarray(feats, np.float32))
    pos_table_np = np.ascontiguousarray(np.asarray(pos_table, np.float32))
    starts, lens, recip, po = _word_ranges(word_lens, pos, seq_len)

    shapes_ok = (
        feats.shape == (B, S, D)
        and pos_table_np.shape == (PV, D)
        and po.shape == (B, W)
        and starts.shape == (B, W)
        and np.asarray(seq_len).shape == (B,)
        and int(po.max()) < PV and int(po.min()) >= 0
    )
    if not shapes_ok or not _concourse_importable():
        return _numpy_fallback(feats, pos_table, word_lens, pos, seq_len)

    blocks_flat, nrows, his, nblk, in_maps, q, slot_of, w0s_all = _prepare(
        feats, pos_table_np, starts, lens, recip, po)
    nc = _build_nc(blocks_flat, nrows, his, nblk)

    res = _run_spmd(nc, in_maps, list(range(N_CORES)))
    LAST_RESULTS = res

    # Accumulate block rows: words split across a block boundary contribute
    # partial sums from two blocks; unused rows are exact zeros.
    out = np.zeros((B, W, D), np.float32)
    for core in range(N_CORES):
        arr = np.asarray(res.results[core]["out"])     # [128, nblk, 768] int8
        dq = (arr.astype(np.float32) / q).transpose(1, 0, 2).reshape(-1, D)
        w0s = w0s_all[core]
        for blk, (bl, a, b) in enumerate(blocks_flat):
            bg = int(slot_of[core, bl])
            w0 = int(w0s[blk])
            if w0 >= W:
                continue
            n = min(int(nrows[blk]), W - w0)
            out[bg, w0:w0 + n] += dq[blk * P:blk * P + n]
    return out


# revision 49
# speedup vs baseline: 1.0977x; 1.0541x over previous
"""TRN2 Bass kernel for nn_CharModel (segment-mean over char ranges + pos embedding).

Strategy (pure data-parallel over batch, 8 cores x 4 batches):
linear fp16 streaming + PE assignment-matmul segment reduce.

  - Valid words tile [0, seq_len) contiguously, so the segment-mean is a
    banded sparse matmul: out[w, :] = sum_{chars p of w} recip_w * feats[p, :].
    Instead of per-word indirect gathers (small descriptors, Pool-engine
    descriptor generation, DVE fold trees), feats is streamed LINEARLY into
    SBUF as [128, 64, 768] fp16 (char-in-tile on partitions) with ~9 KB
    contiguous descriptors, and the reduce runs on the otherwise-idle PE:
      psum[128 words, 768] += A_tile^T @ feats_tile
    where A_tile[p, m] = (wordid[char p] == block_word m) * (q*recip).
  - Word blocks are tile-ALIGNED: greedy runs of 3-4 char tiles whose word
    span stays <= 128 on every core (unified SPMD geometry; batches are
    clustered to core-slots by boundary similarity first). Every char tile
    is matmul'd exactly once. A word straddling a block boundary yields two
    partial sums; the host adds them after dequantization.
  - A_tile matrices are built on-device, one DVE tensor_scalar each:
    (iota == wid_rel[p]) * rq[p], per-partition scalars from tiny host
    tables. rq[p] = q * recip(word(p)) applies each word's 1/len and the
    int8 output scale inside the matmul.
  - The pos embedding is a one-hot matmul accumulated into the same PSUM
    group: psum += OH^T @ pos_table, OH[v, m] = q * (pos[w0+m] == v), set
    only in the first block containing the word. Invalid words keep pos id
    0 and pos_table row 0 is zeros, matching the reference exactly.
  - ACT drains each [128, 768] PSUM block to an int8 SBUF staging tile
    (values bounded by q*(|mean|+|pos|) <= 118 < 127 by construction of q);
    grouped SWDGE writes (4 blocks each) move it to a partition-major DRAM
    tensor. The host transposes, divides by q, and accumulates block rows.
  - Scheduling, learned from traces: blocks interleave round-robin across
    the 4 batches so PE idle stays sub-microsecond (no HAM re-throttle);
    ~28 dummy iota matmuls warm the PE HAM ramp (half->full rate) before
    the real stream; pos matmuls are emitted with a 2-block lag so PSUM
    recycling hides the drain round-trip; const tables load before feats on
    the same queue so their descriptors aren't stuck behind 12.6 MB; output
    writes ride the independent SWDGE queue.
  - HBM traffic per core: 12.6 MB fp16 feats in + 2.3 MB int8 out + ~0.6 MB
    tables, all full-rate descriptors, vs 17.4 MB with gather descriptors
    and pad slots in the v1 gather/fold design (80.9 us -> 59.3 us).
"""

import numpy as np

B, S, W, D, PV = 32, 2048, 512, 768, 64
N_CORES = 8
BPC = B // N_CORES          # batches per core
P = 128
NT = S // P                 # char tiles per batch (16)
WB = W // P                 # word blocks per batch (4)
NBLK = BPC * WB             # psum blocks per core (16)

LAST_RESULTS = None         # BassKernelResults of the most recent run (for test.py)


def _run_spmd(nc, in_maps, core_ids):
    """Indirection point so tests can swap in a simulator."""
    from concourse.bass_utils import run_bass_kernel_spmd
    return run_bass_kernel_spmd(nc, in_maps, core_ids)


def _word_ranges(word_lens, pos, seq_len):
    """Replicate the reference's starts/ends/valid computation in numpy."""
    wl = np.asarray(word_lens, np.int64)
    po = np.asarray(pos, np.int64)
    sl = np.asarray(seq_len, np.int64)
    b, w = wl.shape
    j = np.arange(w)
    next_start = np.concatenate([wl[:, 1:], np.zeros((b, 1), np.int64)], axis=1)
    is_last = (j[None, :] == w - 1) | (next_start == 0)
    starts = wl
    ends = np.where(is_last, sl[:, None], next_start)
    valid = (wl != 0) | (j[None, :] == 0)
    lens = np.where(valid, np.maximum(ends - starts, 0), 0)
    denom = np.maximum(ends - starts, 1).astype(np.float64)
    recip = np.where(valid & (lens > 0), 1.0 / denom, 0.0).astype(np.float32)
    return starts, lens, recip, po


def _numpy_fallback(feats, pos_table, word_lens, pos, seq_len):
    feats = np.asarray(feats, np.float32)
    pos_table = np.asarray(pos_table, np.float32)
    starts, lens, recip, po = _word_ranges(word_lens, pos, seq_len)
    out = np.zeros((feats.shape[0], po.shape[1], feats.shape[2]), np.float32)
    for b in range(out.shape[0]):
        for w in range(out.shape[1]):
            L = int(lens[b, w])
            if L > 0:
                s = int(starts[b, w])
                out[b, w] = feats[b, s:s + L].sum(axis=0) * recip[b, w]
        out[b] += pos_table[po[b]]
    return out


def _concourse_importable():
    try:
        import concourse.bass  # noqa: F401
        return True
    except ImportError:
        import sys
        for p in ("/opt/trn_rl_repo", "/root/.axon_site/_ro/trn_rl_repo"):
            if p not in sys.path:
                sys.path.append(p)
        try:
            import concourse.bass  # noqa: F401
            return True
        except ImportError:
            return False


def _prepare(feats, pos_table_np, starts, lens, recip, po):
    """Host-side layout: wordid map, unified tile windows, per-core tables."""
    amax_f = float(np.abs(feats).max())
    amax_p = float(np.abs(pos_table_np).max()) if pos_table_np.size else 0.0
    q = float(np.float16(127.0 / (amax_f + amax_p + 1e-6)))

    # wordid[b, c] = word owning char c (valid words tile [0, seq_len))
    wordid = np.full((B, S), -10000, np.int32)
    for b in range(B):
        for w in range(W):
            L = int(lens[b, w])
            if L > 0:
                s0 = int(starts[b, w])
                wordid[b, s0:min(s0 + L, S)] = w

    # Assign batches to (core, slot) so batches sharing a slot have similar
    # word-block boundaries: block geometry is unified over cores (SPMD), so
    # clustering by the middle boundary shrinks the unions. Sort by start of
    # word 256 and deal into the 4 slots.
    border = np.array([int(starts[b, W // 2]) for b in range(B)])
    order = np.argsort(border, kind="stable")
    slot_of = np.zeros((N_CORES, BPC), np.int64)
    for g in range(BPC):
        grp = order[g * N_CORES:(g + 1) * N_CORES]
        for c in range(N_CORES):
            slot_of[c, g] = grp[c]

    # number of valid chars per batch (valid words tile [0, nval))
    nval = np.asarray([(wordid[b] >= 0).sum() for b in range(B)])

    # per-slot tile count: max over cores of tiles holding any valid char
    his = np.zeros(BPC, np.int64)
    for bl in range(BPC):
        for c in range(N_CORES):
            bg = int(slot_of[c, bl])
            his[bl] = max(his[bl], -(-int(nval[bg]) // P))

    def span(bg, a, b):
        """word span of chars [a*P, b*P) in batch bg (valid chars only)."""
        nv = int(nval[bg])
        if a * P >= nv:
            return 0
        last = min(b * P, nv) - 1
        return int(wordid[bg, last]) - int(wordid[bg, a * P]) + 1

    # greedy unified blocks per bl: consecutive tiles while every core's
    # word span stays <= 128 (a single tile always fits: span <= 128)
    per_bl = {bl: [] for bl in range(BPC)}
    for bl in range(BPC):
        a = 0
        while a < int(his[bl]):
            b = a + 1
            while b < int(his[bl]) and all(
                    span(int(slot_of[c, bl]), a, b + 1) <= P
                    for c in range(N_CORES)):
                b += 1
            per_bl[bl].append((a, b))
            a = b
    # Interleave blocks round-robin over bl in PAIRS (one ~6-tile DMA chunk
    # feeds two blocks): the PE consumes each batch's work slightly faster
    # than DMA delivers it, and strict bl-major order accumulates that drift
    # into >3us PE idle cliffs at batch boundaries, HAM-throttling the PE to
    # half rate. Round-robin keeps the gaps sub-microsecond.
    blocks_flat = []            # (bl, a, b)
    nrounds = max((len(v) + 1) // 2 for v in per_bl.values())
    for r in range(nrounds):
        for bl in range(BPC):
            for blk in per_bl[bl][2 * r:2 * r + 2]:
                blocks_flat.append((bl,) + blk)
    nrows = [max(span(int(slot_of[c, bl]), a, b) for c in range(N_CORES))
             for (bl, a, b) in blocks_flat]
    nblk = len(blocks_flat)

    iota = np.tile(np.arange(P, dtype=np.float16), (P, 1))
    in_maps = []
    w0s_all = []
    for core in range(N_CORES):
        bgs = [int(slot_of[core, bl]) for bl in range(BPC)]
        fh = feats[bgs].reshape(-1, D).astype(np.float16)         # [8192, 768]
        feats2 = np.ascontiguousarray(
            fh.reshape(BPC * NT, P, D).transpose(1, 0, 2))        # [128, 64, 768]

        # block start word per (core, block); W if the block has no valid
        # chars for this core
        w0s = np.full(nblk, W, np.int64)
        for blk, (bl, a, b) in enumerate(blocks_flat):
            bg = bgs[bl]
            if a * P < int(nval[bg]):
                w0s[blk] = int(wordid[bg, a * P])
        w0s_all.append(w0s)

        # wid_rel per global tile slot, relative to the OWNING block's w0
        widrel = np.full((P, BPC * NT), -20000.0, np.float32)
        for blk, (bl, a, b) in enumerate(blocks_flat):
            bg = bgs[bl]
            for t in range(a, b):
                widrel[:, bl * NT + t] = (
                    wordid[bg, t * P:(t + 1) * P] - w0s[blk]).astype(np.float32)

        rq = np.zeros((P, BPC * NT), np.float32)
        for bl in range(BPC):
            bg = bgs[bl]
            wi = wordid[bg]
            r = np.where(wi >= 0, recip[bg][np.clip(wi, 0, W - 1)] * q, 0.0)
            rq[:, bl * NT:(bl + 1) * NT] = r.reshape(NT, P).T

        # pos one-hot: q at (pos[w], w - w0) in the FIRST block containing w
        oh = np.zeros((PV, nblk * P), np.float16)
        for blk, (bl, a, b) in enumerate(blocks_flat):
            bg = bgs[bl]
            nv = int(nval[bg])
            if a * P >= nv:
                continue
            w0 = int(w0s[blk])
            whigh = int(wordid[bg, min(b * P, nv) - 1])
            wlo = w0
            if a > 0 and a * P - 1 < nv and int(wordid[bg, a * P - 1]) == w0:
                wlo = w0 + 1          # split word: pos set in previous block
            for w in range(wlo, whigh + 1):
                oh[po[bg, w], blk * P + (w - w0)] = np.float16(q)

        in_maps.append({
            "feats2": feats2,
            "pos_tab": pos_table_np.astype(np.float16),
            "iota": iota,
            "widrel": widrel,
            "rq": rq,
            "oh": oh,
        })
    return blocks_flat, nrows, his, nblk, in_maps, q, slot_of, w0s_all


def _build_nc(blocks_flat, nrows, his, nblk):
    from concourse import bacc, mybir
    import concourse.tile as tile

    nc = bacc.Bacc("TRN2", target_bir_lowering=False, debug=False)
    t_feats = nc.dram_tensor("feats2", [P, BPC * NT, D], mybir.dt.float16,
                             kind="ExternalInput")
    t_pos = nc.dram_tensor("pos_tab", [PV, D], mybir.dt.float16,
                           kind="ExternalInput")
    t_iota = nc.dram_tensor("iota", [P, P], mybir.dt.float16,
                            kind="ExternalInput")
    t_widrel = nc.dram_tensor("widrel", [P, BPC * NT], mybir.dt.float32,
                              kind="ExternalInput")
    t_rq = nc.dram_tensor("rq", [P, BPC * NT], mybir.dt.float32,
                          kind="ExternalInput")
    t_oh = nc.dram_tensor("oh", [PV, nblk * P], mybir.dt.float16,
                          kind="ExternalInput")
    # partition-major output: row (p, blk) holds word w0[blk] + p of its
    # block; the host transposes for free. Grouped writes (4 blocks each)
    # amortize the ~1us SWDGE descriptor-generation per DMA.
    t_out = nc.dram_tensor("out", [P, nblk, D], mybir.dt.int8,
                           kind="ExternalOutput")

    with tile.TileContext(nc) as tc:
        PSB = 4     # psum pipeline depth (each buf is bank-aligned: 2 banks)
        with (
            tc.tile_pool(name="const", bufs=1) as cpool,
            tc.tile_pool(name="psum", bufs=PSB, space="PSUM") as ppool,
        ):
            iota_sb = cpool.tile([P, P], mybir.dt.float16)
            widrel_sb = cpool.tile([P, BPC * NT], mybir.dt.float32)
            rq_sb = cpool.tile([P, BPC * NT], mybir.dt.float32)
            oh_sb = cpool.tile([PV, nblk * P], mybir.dt.float16)
            pos_sb = cpool.tile([PV, D], mybir.dt.float16)
            feats_sb = cpool.tile([P, BPC * NT, D], mybir.dt.float16)
            a_sb = cpool.tile([P, BPC * NT * P], mybir.dt.float16)
            osb_all = cpool.tile([P, nblk, D], mybir.dt.int8)

            # consts FIRST on the same queue as feats: their descriptors must
            # not queue behind 12.6 MB of feats in the DMA engines. oh/pos
            # come first so the prefetched pos matmuls can warm the PE early.
            nc.sync.dma_start(out=iota_sb[:], in_=t_iota[:])
            nc.sync.dma_start(out=oh_sb[:], in_=t_oh[:])
            nc.sync.dma_start(out=pos_sb[:], in_=t_pos[:])
            nc.sync.dma_start(out=widrel_sb[:], in_=t_widrel[:])
            nc.sync.dma_start(out=rq_sb[:], in_=t_rq[:])

            # feats chunks aligned to block PAIRS, same round-robin order as
            # the PE consumes blocks (~6-tile chunks amortize DGE overhead)
            seen = {bl: 0 for bl in range(BPC)}
            chunks = []
            for bl, a, b in blocks_flat:
                if b > seen[bl]:
                    chunks.append((bl, seen[bl], b))
                    seen[bl] = b
            merged = []
            for bl, a, b in chunks:
                if (merged and merged[-1][0] == bl
                        and merged[-1][2] == a and merged[-1][1] != 0):
                    merged[-1] = (bl, merged[-1][1], b)
                else:
                    merged.append((bl, a, b))
            for bl, a, b in merged:
                nc.sync.dma_start(
                    out=feats_sb[:, bl * NT + a:bl * NT + b, :],
                    in_=t_feats[:, bl * NT + a:bl * NT + b, :])

            # assignment matrices: A[p, m] = (iota[m] == wid_rel[p]) * rq[p]
            for bl in range(BPC):
                for t in range(int(his[bl])):
                    s = bl * NT + t
                    nc.vector.tensor_scalar(
                        out=a_sb[:, s * P:(s + 1) * P], in0=iota_sb[:, :],
                        scalar1=widrel_sb[:, s:s + 1],
                        scalar2=rq_sb[:, s:s + 1],
                        op0=mybir.AluOpType.is_equal, op1=mybir.AluOpType.mult)

            # HAM warm-up: the PE runs at K=4/8 (half rate) until it has been
            # continuously busy for several microseconds, and any idle gap
            # resets the ramp. Dummy back-to-back matmuls (iota x iota into a
            # scratch PSUM bank) start as soon as the first const lands, so
            # the real stream begins at full rate.
            warm = ppool.tile([P, D], mybir.dt.float32, space="PSUM",
                              tag="ps")
            for _ in range(32):
                nc.tensor.matmul(out=warm[:, 0:128], lhsT=iota_sb[:, :],
                                 rhs=iota_sb[:, :], start=True, stop=True)

            # pos matmuls are prefetched ahead so the PE has work while a
            # block waits for its feats chunk
            psums = {}

            def start_block(blk):
                if blk >= nblk:
                    return
                psum = ppool.tile([P, D], mybir.dt.float32, space="PSUM",
                                  tag="ps")
                lhs = oh_sb[:, blk * P:(blk + 1) * P]
                nc.tensor.matmul(out=psum[:, 0:512], lhsT=lhs,
                                 rhs=pos_sb[:, 0:512], start=True, stop=False)
                nc.tensor.matmul(out=psum[:, 512:D], lhsT=lhs,
                                 rhs=pos_sb[:, 512:D], start=True, stop=False)
                psums[blk] = psum

            # emission lag 2: pos(k+2) is EMITTED after pairs(k), so by the
            # time the PE reaches it, drain(k-2)'s full round-trip (sem +
            # ACT copy + sem, ~2.7us) has completed under two blocks of real
            # work. Emitting earlier makes the PE sit out part of that
            # round-trip between blocks.
            wstart = [0]
            for blk in range(PSB - 2):
                start_block(blk)
            for blk, (bl, a, b) in enumerate(blocks_flat):
                psum = psums.pop(blk)
                for t in range(a, b):
                    s = bl * NT + t
                    asl = a_sb[:, s * P:(s + 1) * P]
                    last = t == b - 1
                    nc.tensor.matmul(out=psum[:, 0:512], lhsT=asl,
                                     rhs=feats_sb[:, s, 0:512],
                                     start=False, stop=last)
                    nc.tensor.matmul(out=psum[:, 512:D], lhsT=asl,
                                     rhs=feats_sb[:, s, 512:D],
                                     start=False, stop=last)
                nc.scalar.activation(out=osb_all[:, blk, :], in_=psum[:, :],
                                     func=mybir.ActivationFunctionType.Copy)
                if (blk % 4 == 3 and blk <= nblk - 5) or blk >= nblk - 2:
                    g = wstart[0]
                    eng = nc.scalar if blk == nblk - 1 else nc.gpsimd
                    eng.dma_start(out=t_out[:, g:blk + 1, :],
                                  in_=osb_all[:, g:blk + 1, :])
                    wstart[0] = blk + 1
                start_block(blk + PSB - 2)
    nc.finalize()
    return nc


def kernel(feats, pos_table, word_lens, pos, seq_len):
    global LAST_RESULTS
    feats = np.ascontiguousarray(np.asarray(feats, np.float32))
    pos_table_np = np.ascontiguousarray(np.asarray(pos_table, np.float32))
    starts, lens, recip, po = _word_ranges(word_lens, pos, seq_len)

    shapes_ok = (
        feats.shape == (B, S, D)
        and pos_table_np.shape == (PV, D)
        and po.shape == (B, W)
        and starts.shape == (B, W)
        and np.asarray(seq_len).shape == (B,)
        and int(po.max()) < PV and int(po.min()) >= 0
    )
    if not shapes_ok or not _concourse_importable():
        return _numpy_fallback(feats, pos_table, word_lens, pos, seq_len)

    blocks_flat, nrows, his, nblk, in_maps, q, slot_of, w0s_all = _prepare(
        feats, pos_table_np, starts, lens, recip, po)
    nc = _build_nc(blocks_flat, nrows, his, nblk)

    res = _run_spmd(nc, in_maps, list(range(N_CORES)))
    LAST_RESULTS = res

    # Accumulate block rows: words split across a block boundary contribute
    # partial sums from two blocks; unused rows are exact zeros.
    out = np.zeros((B, W, D), np.float32)
    for core in range(N_CORES):
        arr = np.asarray(res.results[core]["out"])     # [128, nblk, 768] int8
        dq = (arr.astype(np.float32) / q).transpose(1, 0, 2).reshape(-1, D)
        w0s = w0s_all[core]
        for blk, (bl, a, b) in enumerate(blocks_flat):
            bg = int(slot_of[core, bl])
            w0 = int(w0s[blk])
            if w0 >= W:
                continue
            n = min(int(nrows[blk]), W - w0)
            out[bg, w0:w0 + n] += dq[blk * P:blk * P + n]
    return out
